# revision 61
# baseline (speedup 1.0000x reference)
"""AttnGate sparse-attention block-mask kernel for 8 Trainium2 NeuronCores.

Takes the full unsharded inputs, shards batch x k-head-group across the 8
cores (core c -> batch c//2, k-heads (c%2)*4..+4), runs one SPMD Bass kernel,
and gathers the full [B, Hk, nb] boolean block mask.

v2 structure (vs the v1 baseline):
  - all phase-2 math is done per 128-block chunk in [block, o] layout
    (blocks on partitions), so the rms scale is a per-partition scalar and
    the score is one fused multiply-accumulate.
  - rope and the q path are folded on the host into a single per-(head,
    block) table v[blk, o]; on device the score is
    rsq[blk] * sum_o kc[blk, o] * v[blk, o]  (one scalar_tensor_tensor).
  - max pooling: one DVE tensor_reduce over the in-block axis (f) via a
    transposed 3D access pattern. mean pooling: 64 PSUM-accumulated PE
    transposes (1/64 folded into Wk).
  - scores stream into sc_all[4, 1024]; a merge-chain top-k (extract the
    top-`budget` of running candidates U fresh slabs at chunks 3,5,6,7, all
    4 heads batched on partitions) runs overlapped with the stream; the
    final mask is a single is_ge against the per-head threshold (the
    budget-th largest score).
"""

import json
import math
import os
import sys

import numpy as np

sys.path.insert(0, "/opt/trn_rl_repo")

B, S, HK, D = 4, 65536, 8, 128
BLOCK = 64
NB = S // BLOCK          # 1024 blocks
DG = 128
HQ, G = 32, 4
N_CORES = 8
HEADS_PER_CORE = HK // 2  # 4
CHUNK_BLOCKS = 128        # blocks per pipeline chunk
N_CHUNKS = NB // CHUNK_BLOCKS  # 8
POS_PER_CHUNK = CHUNK_BLOCKS * BLOCK  # 8192 tokens
NEG_MASK = -1e20
SENTINEL = -1e30
EPS = 1e-6
USE_REDUCE_MAX = True

_compiled = {}


# ---------------------------------------------------------------------------
# walrus wait-capacity shim: split multi-wait instructions into single-wait
# NoOp carriers on the same engine (this walrus build accepts one sync wait
# per TPB instruction struct on the failing paths).
# ---------------------------------------------------------------------------
def _split_waits_json(bir_json):
    j = json.loads(bir_json.decode() if isinstance(bir_json, (bytes, bytearray)) else bir_json)
    n = 0
    for f in j.get("functions", []):
        for blk in f.get("blocks", []):
            out = []
            for inst in blk.get("instructions", []):
                si = inst.get("sync_info")
                waits = si.get("on_wait", []) if si else []
                if len(waits) > 1 and inst.get("engine") not in (None, "Unassigned"):
                    for w in waits[:-1]:
                        n += 1
                        out.append({
                            "debug": inst.get("debug", 0),
                            "engine": inst["engine"],
                            "ins": [], "outs": [],
                            "name": "WC-%d" % n,
                            "opcode": "NoOp",
                            "sync_info": {"on_update": [], "on_wait": [w]},
                        })
                    si["on_wait"] = waits[-1:]
                out.append(inst)
            blk["instructions"] = out
    return json.dumps(j).encode()


def _install_waitfix():
    import concourse.bass_utils as bu
    import concourse.bass2jax as b2j
    if getattr(bu, "_attngate_waitfix", False):
        return
    orig = bu.compile_bir_kernel

    def patched(bir_json, tmpdir, neff_name="file.neff"):
        return orig(_split_waits_json(bir_json), tmpdir, neff_name)

    bu.compile_bir_kernel = patched
    b2j.compile_bir_kernel = patched
    bu._attngate_waitfix = True


# ---------------------------------------------------------------------------
# device program
# ---------------------------------------------------------------------------
def _build_program(n_rounds):
    import concourse.bass as bass
    import concourse.mybir as mybir
    from concourse import tile
    from contextlib import ExitStack

    dt = mybir.dt
    f32 = dt.float32
    ALU = mybir.AluOpType
    ACT = mybir.ActivationFunctionType

    budget = n_rounds * 8

    nc = bass.Bass()

    k_d = [nc.dram_tensor("k%d" % h, [S, D], f32, kind="ExternalInput")
           for h in range(HEADS_PER_CORE)]
    wk_d = nc.dram_tensor("wk", [D, HEADS_PER_CORE * 2 * DG], f32, kind="ExternalInput")
    v_d = nc.dram_tensor("vtab", [128, HEADS_PER_CORE * N_CHUNKS * DG], f32, kind="ExternalInput")
    amr_d = nc.dram_tensor("amr", [HEADS_PER_CORE, NB], f32, kind="ExternalInput")
    mtr_d = nc.dram_tensor("mtr", [HEADS_PER_CORE, NB], f32, kind="ExternalInput")
    idn_d = nc.dram_tensor("idn", [128, 128], f32, kind="ExternalInput")
    out_d = nc.dram_tensor("out_mask", [HEADS_PER_CORE, NB], f32, kind="ExternalOutput")

    steps = [(c, h) for c in range(N_CHUNKS) for h in range(HEADS_PER_CORE)]
    NS = len(steps)
    # merge schedule: extract top-budget of slabs 0..4 after chunk 4
    # (trickled); slabs 5..7 join at the end.
    MERGE1_C = 4

    with tile.TileContext(nc) as tc, ExitStack() as ctx:
        consts = ctx.enter_context(tc.tile_pool(name="consts", bufs=1))
        chunks = ctx.enter_context(tc.tile_pool(name="chunks", bufs=3))
        trees = ctx.enter_context(tc.tile_pool(name="trees", bufs=1))
        pooled = ctx.enter_context(tc.tile_pool(name="pooled", bufs=2))
        ph2 = ctx.enter_context(tc.tile_pool(name="ph2", bufs=2))
        ph3 = ctx.enter_context(tc.tile_pool(name="ph3", bufs=3))
        candp = ctx.enter_context(tc.tile_pool(name="candp", bufs=1))
        psA_p = ctx.enter_context(tc.tile_pool(name="psA", bufs=2, space="PSUM"))
        psM_p = ctx.enter_context(tc.tile_pool(name="psM", bufs=2, space="PSUM"))
        psKC_p = ctx.enter_context(tc.tile_pool(name="psKC", bufs=2, space="PSUM"))
        psS_p = ctx.enter_context(tc.tile_pool(name="psS", bufs=2, space="PSUM"))

        kts = {}
        def emit_dma(i):
            c, h = steps[i]
            kt = chunks.tile([128, BLOCK, D], f32, tag="kt")
            nc.sync.dma_start(
                kt[:, :, :],
                k_d[h][c * POS_PER_CHUNK:(c + 1) * POS_PER_CHUNK, :]
                .rearrange("(p f) d -> p f d", p=128))
            kts[i] = kt

        # first chunk DMA goes out first; consts trickle in behind it in
        # order of first use so they do not steal bandwidth from chunk 0.
        emit_dma(0)
        idn = consts.tile([128, 128], f32)
        nc.sync.dma_start(idn[:], idn_d[:, :])
        emit_dma(1)
        wk = consts.tile([128, HEADS_PER_CORE * 2 * DG], f32)
        nc.sync.dma_start(wk[:], wk_d[:, :])
        emit_dma(2)
        amr = consts.tile([HEADS_PER_CORE, NB], f32)
        nc.sync.dma_start(amr[:], amr_d[:, :])
        mtr = consts.tile([HEADS_PER_CORE, NB], f32)
        nc.sync.dma_start(mtr[:], mtr_d[:, :])
        vtab = consts.tile([128, HEADS_PER_CORE * N_CHUNKS * DG], f32)

        sc_all = consts.tile([HEADS_PER_CORE, NB], f32)
        epsc = consts.tile([128, 1], f32)
        nc.vector.memset(epsc[:], EPS)

        state = {}
        s4_by_c = {}
        merge_backlog = []
        merge_M = [None, None]

        def emit_front(i):
            c, h = steps[i]
            kt = kts[i]
            # max pool over f -> [blk, d]: pairwise tree, contiguous slices
            prev, size = kt, BLOCK
            lvl = 0
            while size > 1:
                half = size // 2
                pool = trees if half > 1 else pooled
                nxt = pool.tile([128, half, D], f32, tag="tr%d" % lvl)
                nc.vector.tensor_tensor(nxt[:, :, :], prev[:, 0:half, :],
                                        prev[:, half:size, :], ALU.max)
                prev, size = nxt, half
                lvl += 1
            # mean: 64 accumulated PE transposes -> [d, blk]
            psA = psA_p.tile([128, 128], f32, tag="psA")
            for f in range(BLOCK):
                nc.tensor.matmul(psA[:], kt[:, f, :], idn[:],
                                 is_transpose=True,
                                 start=(f == 0), stop=(f == BLOCK - 1))
            meanT = pooled.tile([128, 128], f32, tag="meanT")
            nc.scalar.copy(meanT[:], psA[:])
            state[("front", i)] = (prev, meanT)

        def emit_mid(i):
            c, h = steps[i]
            mxP, meanT = state.pop(("front", i))
            if h == 0:
                # sr[:, 0:4] = unscaled score columns, sr[:, 4:8] = rms roots
                s4_by_c[c] = ph3.tile([128, 2 * HEADS_PER_CORE], f32, tag="sr",
                                      name="sr")
            sr = s4_by_c[c]
            psM = psM_p.tile([128, 128], f32, tag="psM")
            nc.tensor.matmul(psM[:], mxP[:, 0, :], idn[:],
                             is_transpose=True, start=True, stop=True)
            maxT = pooled.tile([128, 128], f32, tag="maxT")
            nc.scalar.copy(maxT[:], psM[:])
            # projection: kc[blk, o] (pooled_T as stationary)
            psKC = psKC_p.tile([128, 128], f32, tag="psKC")
            nc.tensor.matmul(psKC[:], meanT[:],
                             wk[:, (h * 2) * DG:(h * 2 + 1) * DG],
                             start=True, stop=False)
            nc.tensor.matmul(psKC[:], maxT[:],
                             wk[:, (h * 2 + 1) * DG:(h * 2 + 2) * DG],
                             start=False, stop=True)
            kc = ph3.tile([128, 128], f32, tag="kc")
            nc.scalar.copy(kc[:], psKC[:])
            # rms root: sr[:, 4+h] = sqrt(mean(kc^2) + eps), all on Act
            sq = ph2.tile([128, 128], f32, tag="sq")
            rss = ph2.tile([128, 1], f32, tag="rss")
            nc.scalar.activation(sq[:], psKC[:], ACT.Square, accum_out=rss[:])
            r2 = ph2.tile([128, 1], f32, tag="r2")
            nc.scalar.activation(r2[:], rss[:], ACT.Identity,
                                 bias=epsc[:], scale=1.0 / DG)
            nc.scalar.activation(sr[:, 4 + h:5 + h], r2[:], ACT.Sqrt)
            state[i] = kc

        def emit_back(i):
            c, h = steps[i]
            kc = state.pop(i)
            sr = s4_by_c[c]
            vi = (h * N_CHUNKS + c) * DG
            j1 = ph2.tile([128, 128], f32, tag="j1")
            nc.vector.scalar_tensor_tensor(j1[:], kc[:], 1.0,
                                           vtab[:, vi:vi + DG],
                                           ALU.mult, ALU.mult,
                                           accum_out=sr[:, h:h + 1])

        def emit_chunk_back_a(c):
            # transpose scores+roots, start the divide
            sr = s4_by_c.pop(c)
            psS = psS_p.tile([HEADS_PER_CORE, 256], f32, tag="psS")
            nc.tensor.matmul(psS[:, 0:128], sr[:, 0:4], idn[:],
                             is_transpose=True, start=True, stop=True)
            nc.tensor.matmul(psS[:, 128:256], sr[:, 4:8], idn[:],
                             is_transpose=True, start=True, stop=True)
            rr = ph2.tile([HEADS_PER_CORE, 128], f32, tag="rr")
            nc.vector.reciprocal(rr[:], psS[:, 128:256])
            state[("psS", c)] = (psS, rr)

        def emit_chunk_back(c):
            # finish: divide, mask, store the score rows
            psS, rr = state.pop(("psS", c))
            srow = ph2.tile([HEADS_PER_CORE, 128], f32, tag="srow")
            nc.vector.tensor_tensor(srow[:], psS[:, 0:128], rr[:], ALU.mult)
            sl = slice(c * CHUNK_BLOCKS, (c + 1) * CHUNK_BLOCKS)
            nc.vector.tensor_tensor(srow[:], srow[:], amr[:, sl], ALU.mult)
            nc.vector.tensor_tensor(srow[:], srow[:], mtr[:, sl], ALU.add)
            nc.scalar.copy(sc_all[:, sl], srow[:])
            if c == MERGE1_C:
                queue_merge(None, 0, c + 1)

        def queue_merge(prev_M, c_lo, c_hi):
            # extract top-budget of [prev_M | slabs c_lo..c_hi-1], trickled
            nslab = (c_hi - c_lo) * CHUNK_BLOCKS
            mw = (budget if prev_M is not None else 0) + nslab
            midx = 0 if prev_M is None else 1
            work = candp.tile([HEADS_PER_CORE, mw], f32, tag="w%d" % midx,
                              name="mwork")
            off = 0
            if prev_M is not None:
                nc.scalar.copy(work[:, 0:budget], prev_M[:])
                off = budget
            nc.scalar.copy(work[:, off:off + nslab],
                           sc_all[:, c_lo * CHUNK_BLOCKS:c_hi * CHUNK_BLOCKS])
            M = candp.tile([HEADS_PER_CORE, budget], f32, tag="c%d" % midx,
                           name="mcand")
            merge_M[midx] = M

            def round_thunk(r):
                def go():
                    m8 = M[:, r * 8:(r + 1) * 8]
                    nc.vector.max(m8, work[:])
                    nc.vector.match_replace(work[:], m8, work[:], SENTINEL)
                return go
            for r in range(n_rounds):
                merge_backlog.append(round_thunk(r))

        # ---- software-pipelined emission --------------------------------
        CB_SKEW = 6   # chunk-back for chunk c runs at slots 4c+CB_SKEW (+1)
        for slot in range(NS + CB_SKEW + 2):
            if 3 <= slot < NS:
                emit_dma(slot)
            if slot == 2:
                nc.sync.dma_start(vtab[:], v_d[:, :])
            if slot < NS:
                emit_front(slot)
            if 1 <= slot <= NS:
                emit_mid(slot - 1)
            if 2 <= slot <= NS + 1:
                emit_back(slot - 2)
            if slot >= CB_SKEW and (slot - CB_SKEW) % 4 == 0:
                cb = (slot - CB_SKEW) // 4
                if cb < N_CHUNKS:
                    emit_chunk_back_a(cb)
            if slot >= CB_SKEW + 1 and (slot - CB_SKEW - 1) % 4 == 0:
                cb = (slot - CB_SKEW - 1) // 4
                if cb < N_CHUNKS:
                    emit_chunk_back(cb)
            npop = 2 if len(merge_backlog) > (NS + CB_SKEW + 1 - slot) else 1
            for _ in range(npop):
                if merge_backlog:
                    merge_backlog.pop(0)()

        while merge_backlog:
            merge_backlog.pop(0)()

        # ---- final merge: [merge_M[0] | slabs 5..7] ---------------------
        rem = (N_CHUNKS - MERGE1_C - 1) * CHUNK_BLOCKS
        fw = budget + rem
        fwork = candp.tile([HEADS_PER_CORE, fw], f32, tag="wfin")
        nc.scalar.copy(fwork[:, 0:budget], merge_M[0][:])
        nc.scalar.copy(fwork[:, budget:fw],
                       sc_all[:, (MERGE1_C + 1) * CHUNK_BLOCKS:NB])
        fM = candp.tile([HEADS_PER_CORE, budget], f32, tag="cfin")
        for r in range(n_rounds):
            m8 = fM[:, r * 8:(r + 1) * 8]
            nc.vector.max(m8, fwork[:])
            nc.vector.match_replace(fwork[:], m8, fwork[:], SENTINEL)

        # ---- final threshold + mask ------------------------------------
        t_ap = fM[:, budget - 1:budget]
        maskf = consts.tile([HEADS_PER_CORE, NB], f32)
        nc.vector.tensor_scalar(maskf[:], sc_all[:], t_ap, None, ALU.is_ge)
        nc.sync.dma_start(out_d[:, :], maskf[:])

    return nc


def _rot_w(w):
    return np.concatenate([w[DG // 2:], w[:DG // 2]])


def _rmsnorm_np(x, w, eps=1e-6):
    v = np.mean(np.square(x), axis=-1, keepdims=True)
    return x / np.sqrt(v + eps) * w


def kernel(k, q, Wq, Wk, qnorm_w, knorm_w, cos_q, sin_q, cos_k, sin_k,
           attention_mask, block_budget):
    _install_waitfix()
    from concourse.bass_utils import run_bass_kernel_spmd

    k = np.asarray(k, dtype=np.float32)
    q = np.asarray(q, dtype=np.float32)
    Wq = np.asarray(Wq, dtype=np.float32)
    Wk = np.asarray(Wk, dtype=np.float32)
    qnorm_w = np.asarray(qnorm_w, dtype=np.float32)
    knorm_w = np.asarray(knorm_w, dtype=np.float32)
    cos_q = np.asarray(cos_q, dtype=np.float32)
    sin_q = np.asarray(sin_q, dtype=np.float32)
    cos_k = np.asarray(cos_k, dtype=np.float32)
    sin_k = np.asarray(sin_k, dtype=np.float32)
    am = np.asarray(attention_mask).astype(bool)
    budget = int(block_budget)
    assert budget % 8 == 0 and 0 < budget <= 512
    n_rounds = budget // 8

    scale = 1.0 / math.sqrt(DG)

    key = (n_rounds,)
    if key not in _compiled:
        _compiled[key] = _build_program(n_rounds)
    nc = _compiled[key]

    idn_np = np.eye(128, dtype=np.float32)

    # ---- host q path: qv = rope(rmsnorm(q @ Wq)) * scale ----------------
    qg = q[:, 0].reshape(B, HK, G, D)
    qp = np.einsum('bkgi,kgio->bko', qg, Wq)
    qp = _rmsnorm_np(qp, qnorm_w)
    cq = cos_q[:, 0]  # [B, Dg]
    sq = sin_q[:, 0]
    rot = np.concatenate([-qp[..., DG // 2:], qp[..., :DG // 2]], axis=-1)
    qv = (qp * cq[:, None, :] + rot * sq[:, None, :]) * scale  # [B, Hk, Dg]
    qv = qv.astype(np.float32)

    # folded cos/sin (k): ck = cos*w ; sk = sin*rot_w(w) with sign on 1st half
    ckw = cos_k * knorm_w[None, None, :]                       # [B, nb, Dg]
    skw = sin_k * _rot_w(knorm_w)[None, None, :]
    skw = skw.copy()
    skw[:, :, :DG // 2] *= -1.0

    # v table: v[b, h, blk, o] = ck[blk,o]*qv[h,o] + swap_halves(sk[blk]*qv[h])
    # so that  score = sum_o kc[blk,o] * v[blk,o]  reproduces
    #          sum_o rope(kc)[blk,o] * qv[o].
    vt = (ckw[:, None, :, :] * qv[:, :, None, :])              # [B, Hk, nb, Dg]
    su = skw[:, None, :, :] * qv[:, :, None, :]
    vt = vt + np.concatenate([su[..., DG // 2:], su[..., :DG // 2]], axis=-1)
    vt = vt.astype(np.float32)

    in_maps = []
    for core in range(N_CORES):
        b = core // 2
        h0 = (core % 2) * HEADS_PER_CORE
        heads = list(range(h0, h0 + HEADS_PER_CORE))
        im = {}
        for i, h in enumerate(heads):
            im["k%d" % i] = np.ascontiguousarray(k[b, :, h, :])
        # wk: [d, (h, 2, o)]; mean part scaled by 1/64
        wk_prep = np.empty((D, HEADS_PER_CORE, 2, DG), dtype=np.float32)
        for i, h in enumerate(heads):
            wk_prep[:, i, 0, :] = Wk[h, :D, :] / BLOCK
            wk_prep[:, i, 1, :] = Wk[h, D:, :]
        im["wk"] = wk_prep.reshape(D, HEADS_PER_CORE * 2 * DG)
        # v table: [p, (h, chunk, o)] with p = block index within chunk
        v_prep = np.empty((128, HEADS_PER_CORE, N_CHUNKS, DG), dtype=np.float32)
        for i, h in enumerate(heads):
            for cc in range(N_CHUNKS):
                v_prep[:, i, cc, :] = vt[b, h, cc * CHUNK_BLOCKS:(cc + 1) * CHUNK_BLOCKS, :]
        im["vtab"] = v_prep.reshape(128, HEADS_PER_CORE * N_CHUNKS * DG)
        # mask folds in row layout [h, nb]
        amr_prep = am[b, heads, :].astype(np.float32)
        im["amr"] = amr_prep
        im["mtr"] = ((1.0 - amr_prep) * NEG_MASK).astype(np.float32)
        im["idn"] = idn_np
        in_maps.append(im)

    res = run_bass_kernel_spmd(nc, in_maps, core_ids=list(range(N_CORES)),
                               trace=bool(int(os.environ.get("ATTNGATE_TRACE", "0"))))
    kernel.last_result = res

    sel = np.zeros((B, HK, NB), dtype=bool)
    for core in range(N_CORES):
        b = core // 2
        h0 = (core % 2) * HEADS_PER_CORE
        sel[b, h0:h0 + HEADS_PER_CORE, :] = res.results[core]["out_mask"] != 0.0
    mask = sel & am
    mask[:, :, -1] = True
    return mask


# revision 64
# speedup vs baseline: 1.0255x; 1.0255x over previous
"""AttnGate sparse-attention block-mask kernel for 8 Trainium2 NeuronCores.

Takes the full unsharded inputs, shards batch x k-head-group across the 8
cores (core c -> batch c//2, k-heads (c%2)*4..+4), runs one SPMD Bass kernel,
and gathers the full [B, Hk, nb] boolean block mask.

v2 structure (vs the v1 baseline):
  - all phase-2 math is done per 128-block chunk in [block, o] layout
    (blocks on partitions), so the rms scale is a per-partition scalar and
    the score is one fused multiply-accumulate.
  - rope and the q path are folded on the host into a single per-(head,
    block) table v[blk, o]; on device the score is
    rsq[blk] * sum_o kc[blk, o] * v[blk, o]  (one scalar_tensor_tensor).
  - max pooling: one DVE tensor_reduce over the in-block axis (f) via a
    transposed 3D access pattern. mean pooling: 64 PSUM-accumulated PE
    transposes (1/64 folded into Wk).
  - scores stream into sc_all[4, 1024]; a merge-chain top-k (extract the
    top-`budget` of running candidates U fresh slabs at chunks 3,5,6,7, all
    4 heads batched on partitions) runs overlapped with the stream; the
    final mask is a single is_ge against the per-head threshold (the
    budget-th largest score).
"""

import json
import math
import os
import sys

import numpy as np

sys.path.insert(0, "/opt/trn_rl_repo")

B, S, HK, D = 4, 65536, 8, 128
BLOCK = 64
NB = S // BLOCK          # 1024 blocks
DG = 128
HQ, G = 32, 4
N_CORES = 8
HEADS_PER_CORE = HK // 2  # 4
CHUNK_BLOCKS = 128        # blocks per pipeline chunk
N_CHUNKS = NB // CHUNK_BLOCKS  # 8
POS_PER_CHUNK = CHUNK_BLOCKS * BLOCK  # 8192 tokens
NEG_MASK = -1e20
SENTINEL = -1e30
EPS = 1e-6
USE_REDUCE_MAX = True

_compiled = {}


# ---------------------------------------------------------------------------
# walrus wait-capacity shim: split multi-wait instructions into single-wait
# NoOp carriers on the same engine (this walrus build accepts one sync wait
# per TPB instruction struct on the failing paths).
# ---------------------------------------------------------------------------
def _split_waits_json(bir_json):
    j = json.loads(bir_json.decode() if isinstance(bir_json, (bytes, bytearray)) else bir_json)
    n = 0
    for f in j.get("functions", []):
        for blk in f.get("blocks", []):
            out = []
            for inst in blk.get("instructions", []):
                si = inst.get("sync_info")
                waits = si.get("on_wait", []) if si else []
                if len(waits) > 1 and inst.get("engine") not in (None, "Unassigned"):
                    for w in waits[:-1]:
                        n += 1
                        out.append({
                            "debug": inst.get("debug", 0),
                            "engine": inst["engine"],
                            "ins": [], "outs": [],
                            "name": "WC-%d" % n,
                            "opcode": "NoOp",
                            "sync_info": {"on_update": [], "on_wait": [w]},
                        })
                    si["on_wait"] = waits[-1:]
                out.append(inst)
            blk["instructions"] = out
    return json.dumps(j).encode()


def _install_waitfix():
    import concourse.bass_utils as bu
    import concourse.bass2jax as b2j
    if getattr(bu, "_attngate_waitfix", False):
        return
    orig = bu.compile_bir_kernel

    def patched(bir_json, tmpdir, neff_name="file.neff"):
        return orig(_split_waits_json(bir_json), tmpdir, neff_name)

    bu.compile_bir_kernel = patched
    b2j.compile_bir_kernel = patched
    bu._attngate_waitfix = True


# ---------------------------------------------------------------------------
# device program
# ---------------------------------------------------------------------------
def _build_program(n_rounds):
    import concourse.bass as bass
    import concourse.mybir as mybir
    from concourse import tile
    from contextlib import ExitStack

    dt = mybir.dt
    f32 = dt.float32
    ALU = mybir.AluOpType
    ACT = mybir.ActivationFunctionType

    budget = n_rounds * 8

    nc = bass.Bass()

    k_d = [nc.dram_tensor("k%d" % h, [S, D], f32, kind="ExternalInput")
           for h in range(HEADS_PER_CORE)]
    wk_d = nc.dram_tensor("wk", [D, HEADS_PER_CORE * 2 * DG], f32, kind="ExternalInput")
    v_d = nc.dram_tensor("vtab", [128, HEADS_PER_CORE * N_CHUNKS * DG], f32, kind="ExternalInput")
    amr_d = nc.dram_tensor("amr", [HEADS_PER_CORE, NB], f32, kind="ExternalInput")
    mtr_d = nc.dram_tensor("mtr", [HEADS_PER_CORE, NB], f32, kind="ExternalInput")
    idn_d = nc.dram_tensor("idn", [128, 128], f32, kind="ExternalInput")
    out_d = nc.dram_tensor("out_mask", [HEADS_PER_CORE, NB], f32, kind="ExternalOutput")

    steps = [(c, h) for c in range(N_CHUNKS) for h in range(HEADS_PER_CORE)]
    NS = len(steps)
    # merge schedule: extract top-budget of slabs 0..3 after chunk 3, fold
    # slabs 4..5 in after chunk 5 (both trickled), join 6..7 at the end.
    MERGE1_C, MERGE2_C = 3, 5

    with tile.TileContext(nc) as tc, ExitStack() as ctx:
        consts = ctx.enter_context(tc.tile_pool(name="consts", bufs=1))
        chunks = ctx.enter_context(tc.tile_pool(name="chunks", bufs=3))
        trees = ctx.enter_context(tc.tile_pool(name="trees", bufs=1))
        pooled = ctx.enter_context(tc.tile_pool(name="pooled", bufs=2))
        ph2 = ctx.enter_context(tc.tile_pool(name="ph2", bufs=2))
        ph3 = ctx.enter_context(tc.tile_pool(name="ph3", bufs=3))
        candp = ctx.enter_context(tc.tile_pool(name="candp", bufs=1))
        psA_p = ctx.enter_context(tc.tile_pool(name="psA", bufs=2, space="PSUM"))
        psM_p = ctx.enter_context(tc.tile_pool(name="psM", bufs=2, space="PSUM"))
        psKC_p = ctx.enter_context(tc.tile_pool(name="psKC", bufs=2, space="PSUM"))
        psS_p = ctx.enter_context(tc.tile_pool(name="psS", bufs=2, space="PSUM"))

        kts = {}
        def emit_dma(i):
            c, h = steps[i]
            kt = chunks.tile([128, BLOCK, D], f32, tag="kt")
            nc.sync.dma_start(
                kt[:, :, :],
                k_d[h][c * POS_PER_CHUNK:(c + 1) * POS_PER_CHUNK, :]
                .rearrange("(p f) d -> p f d", p=128))
            kts[i] = kt

        # first chunk DMA goes out first; consts trickle in behind it in
        # order of first use so they do not steal bandwidth from chunk 0.
        emit_dma(0)
        idn = consts.tile([128, 128], f32)
        nc.sync.dma_start(idn[:], idn_d[:, :])
        emit_dma(1)
        wk = consts.tile([128, HEADS_PER_CORE * 2 * DG], f32)
        nc.sync.dma_start(wk[:], wk_d[:, :])
        emit_dma(2)
        amr = consts.tile([HEADS_PER_CORE, NB], f32)
        nc.sync.dma_start(amr[:], amr_d[:, :])
        mtr = consts.tile([HEADS_PER_CORE, NB], f32)
        nc.sync.dma_start(mtr[:], mtr_d[:, :])
        vtab = consts.tile([128, HEADS_PER_CORE * N_CHUNKS * DG], f32)

        sc_all = consts.tile([HEADS_PER_CORE, NB], f32)
        epsc = consts.tile([128, 1], f32)
        nc.vector.memset(epsc[:], EPS)

        state = {}
        s4_by_c = {}
        merge_backlog = []
        merge_M = [None, None]

        def emit_front(i):
            c, h = steps[i]
            kt = kts[i]
            # max pool over f -> [blk, d]: pairwise tree, contiguous slices
            prev, size = kt, BLOCK
            lvl = 0
            while size > 1:
                half = size // 2
                pool = trees if half > 1 else pooled
                nxt = pool.tile([128, half, D], f32, tag="tr%d" % lvl)
                nc.vector.tensor_tensor(nxt[:, :, :], prev[:, 0:half, :],
                                        prev[:, half:size, :], ALU.max)
                prev, size = nxt, half
                lvl += 1
            # mean: 64 accumulated PE transposes -> [d, blk]
            psA = psA_p.tile([128, 128], f32, tag="psA")
            for f in range(BLOCK):
                nc.tensor.matmul(psA[:], kt[:, f, :], idn[:],
                                 is_transpose=True,
                                 start=(f == 0), stop=(f == BLOCK - 1))
            meanT = pooled.tile([128, 128], f32, tag="meanT")
            nc.scalar.copy(meanT[:], psA[:])
            state[("front", i)] = (prev, meanT)

        def emit_mid(i):
            c, h = steps[i]
            mxP, meanT = state.pop(("front", i))
            if h == 0:
                # sr[:, 0:4] = unscaled score columns, sr[:, 4:8] = rms roots
                s4_by_c[c] = ph3.tile([128, 2 * HEADS_PER_CORE], f32, tag="sr",
                                      name="sr")
            sr = s4_by_c[c]
            psM = psM_p.tile([128, 128], f32, tag="psM")
            nc.tensor.matmul(psM[:], mxP[:, 0, :], idn[:],
                             is_transpose=True, start=True, stop=True)
            maxT = pooled.tile([128, 128], f32, tag="maxT")
            nc.scalar.copy(maxT[:], psM[:])
            # projection: kc[blk, o] (pooled_T as stationary)
            psKC = psKC_p.tile([128, 128], f32, tag="psKC")
            nc.tensor.matmul(psKC[:], meanT[:],
                             wk[:, (h * 2) * DG:(h * 2 + 1) * DG],
                             start=True, stop=False)
            nc.tensor.matmul(psKC[:], maxT[:],
                             wk[:, (h * 2 + 1) * DG:(h * 2 + 2) * DG],
                             start=False, stop=True)
            kc = ph3.tile([128, 128], f32, tag="kc")
            nc.scalar.copy(kc[:], psKC[:])
            # rms root: sr[:, 4+h] = sqrt(mean(kc^2) + eps), all on Act
            sq = ph2.tile([128, 128], f32, tag="sq")
            rss = ph2.tile([128, 1], f32, tag="rss")
            nc.scalar.activation(sq[:], psKC[:], ACT.Square, accum_out=rss[:])
            r2 = ph2.tile([128, 1], f32, tag="r2")
            nc.scalar.activation(r2[:], rss[:], ACT.Identity,
                                 bias=epsc[:], scale=1.0 / DG)
            nc.scalar.activation(sr[:, 4 + h:5 + h], r2[:], ACT.Sqrt)
            state[i] = kc

        def emit_back(i):
            c, h = steps[i]
            kc = state.pop(i)
            sr = s4_by_c[c]
            vi = (h * N_CHUNKS + c) * DG
            j1 = ph2.tile([128, 128], f32, tag="j1")
            nc.vector.scalar_tensor_tensor(j1[:], kc[:], 1.0,
                                           vtab[:, vi:vi + DG],
                                           ALU.mult, ALU.mult,
                                           accum_out=sr[:, h:h + 1])

        def emit_chunk_back_a(c):
            # transpose scores+roots, start the divide
            sr = s4_by_c.pop(c)
            psS = psS_p.tile([HEADS_PER_CORE, 256], f32, tag="psS")
            nc.tensor.matmul(psS[:, 0:128], sr[:, 0:4], idn[:],
                             is_transpose=True, start=True, stop=True)
            nc.tensor.matmul(psS[:, 128:256], sr[:, 4:8], idn[:],
                             is_transpose=True, start=True, stop=True)
            rr = ph2.tile([HEADS_PER_CORE, 128], f32, tag="rr")
            nc.vector.reciprocal(rr[:], psS[:, 128:256])
            state[("psS", c)] = (psS, rr)

        def emit_chunk_back(c):
            # finish: divide, mask, store the score rows
            psS, rr = state.pop(("psS", c))
            srow = ph2.tile([HEADS_PER_CORE, 128], f32, tag="srow")
            nc.vector.tensor_tensor(srow[:], psS[:, 0:128], rr[:], ALU.mult)
            sl = slice(c * CHUNK_BLOCKS, (c + 1) * CHUNK_BLOCKS)
            nc.vector.tensor_tensor(srow[:], srow[:], amr[:, sl], ALU.mult)
            nc.vector.tensor_tensor(srow[:], srow[:], mtr[:, sl], ALU.add)
            nc.scalar.copy(sc_all[:, sl], srow[:])
            if c == MERGE1_C:
                queue_merge(None, 0, c + 1)
            elif c == MERGE2_C:
                # defer until merge1's rounds are fully emitted
                merge_backlog.append(
                    lambda: queue_merge(merge_M[0], MERGE1_C + 1, MERGE2_C + 1))

        def queue_merge(prev_M, c_lo, c_hi):
            # extract top-budget of [prev_M | slabs c_lo..c_hi-1], trickled
            nslab = (c_hi - c_lo) * CHUNK_BLOCKS
            mw = (budget if prev_M is not None else 0) + nslab
            midx = 0 if prev_M is None else 1
            work = candp.tile([HEADS_PER_CORE, mw], f32, tag="w%d" % midx,
                              name="mwork")
            off = 0
            if prev_M is not None:
                nc.scalar.copy(work[:, 0:budget], prev_M[:])
                off = budget
            nc.scalar.copy(work[:, off:off + nslab],
                           sc_all[:, c_lo * CHUNK_BLOCKS:c_hi * CHUNK_BLOCKS])
            M = candp.tile([HEADS_PER_CORE, budget], f32, tag="c%d" % midx,
                           name="mcand")
            merge_M[midx] = M

            def round_thunk(r):
                def go():
                    m8 = M[:, r * 8:(r + 1) * 8]
                    nc.vector.max(m8, work[:])
                    nc.vector.match_replace(work[:], m8, work[:], SENTINEL)
                return go
            for r in range(n_rounds):
                merge_backlog.append(round_thunk(r))

        # ---- software-pipelined emission --------------------------------
        CB_SKEW = 6   # chunk-back for chunk c runs at slots 4c+CB_SKEW (+1)
        for slot in range(NS + CB_SKEW + 2):
            if 3 <= slot < NS:
                emit_dma(slot)
            if slot == 2:
                nc.sync.dma_start(vtab[:], v_d[:, :])
            if slot < NS:
                emit_front(slot)
            if 1 <= slot <= NS:
                emit_mid(slot - 1)
            if 2 <= slot <= NS + 1:
                emit_back(slot - 2)
            if slot >= CB_SKEW and (slot - CB_SKEW) % 4 == 0:
                cb = (slot - CB_SKEW) // 4
                if cb < N_CHUNKS:
                    emit_chunk_back_a(cb)
            if slot >= CB_SKEW + 1 and (slot - CB_SKEW - 1) % 4 == 0:
                cb = (slot - CB_SKEW - 1) // 4
                if cb < N_CHUNKS:
                    emit_chunk_back(cb)
            npop = 2 if len(merge_backlog) > (NS + CB_SKEW + 1 - slot) else 1
            for _ in range(npop):
                if merge_backlog:
                    merge_backlog.pop(0)()

        while merge_backlog:
            merge_backlog.pop(0)()

        # ---- final merge: [merge_M[1] | slabs 6..7] ---------------------
        rem = (N_CHUNKS - MERGE2_C - 1) * CHUNK_BLOCKS
        fw = budget + rem
        fwork = candp.tile([HEADS_PER_CORE, fw], f32, tag="wfin")
        nc.scalar.copy(fwork[:, 0:budget], merge_M[1][:])
        nc.scalar.copy(fwork[:, budget:fw],
                       sc_all[:, (MERGE2_C + 1) * CHUNK_BLOCKS:NB])
        fM = candp.tile([HEADS_PER_CORE, budget], f32, tag="cfin")
        for r in range(n_rounds):
            m8 = fM[:, r * 8:(r + 1) * 8]
            nc.vector.max(m8, fwork[:])
            nc.vector.match_replace(fwork[:], m8, fwork[:], SENTINEL)

        # ---- final threshold + mask ------------------------------------
        t_ap = fM[:, budget - 1:budget]
        maskf = consts.tile([HEADS_PER_CORE, NB], f32)
        nc.vector.tensor_scalar(maskf[:], sc_all[:], t_ap, None, ALU.is_ge)
        nc.sync.dma_start(out_d[:, :], maskf[:])

    return nc


def _rot_w(w):
    return np.concatenate([w[DG // 2:], w[:DG // 2]])


def _rmsnorm_np(x, w, eps=1e-6):
    v = np.mean(np.square(x), axis=-1, keepdims=True)
    return x / np.sqrt(v + eps) * w


def kernel(k, q, Wq, Wk, qnorm_w, knorm_w, cos_q, sin_q, cos_k, sin_k,
           attention_mask, block_budget):
    _install_waitfix()
    from concourse.bass_utils import run_bass_kernel_spmd

    k = np.asarray(k, dtype=np.float32)
    q = np.asarray(q, dtype=np.float32)
    Wq = np.asarray(Wq, dtype=np.float32)
    Wk = np.asarray(Wk, dtype=np.float32)
    qnorm_w = np.asarray(qnorm_w, dtype=np.float32)
    knorm_w = np.asarray(knorm_w, dtype=np.float32)
    cos_q = np.asarray(cos_q, dtype=np.float32)
    sin_q = np.asarray(sin_q, dtype=np.float32)
    cos_k = np.asarray(cos_k, dtype=np.float32)
    sin_k = np.asarray(sin_k, dtype=np.float32)
    am = np.asarray(attention_mask).astype(bool)
    budget = int(block_budget)
    assert budget % 8 == 0 and 0 < budget <= 512
    n_rounds = budget // 8

    scale = 1.0 / math.sqrt(DG)

    key = (n_rounds,)
    if key not in _compiled:
        _compiled[key] = _build_program(n_rounds)
    nc = _compiled[key]

    idn_np = np.eye(128, dtype=np.float32)

    # ---- host q path: qv = rope(rmsnorm(q @ Wq)) * scale ----------------
    qg = q[:, 0].reshape(B, HK, G, D)
    qp = np.einsum('bkgi,kgio->bko', qg, Wq)
    qp = _rmsnorm_np(qp, qnorm_w)
    cq = cos_q[:, 0]  # [B, Dg]
    sq = sin_q[:, 0]
    rot = np.concatenate([-qp[..., DG // 2:], qp[..., :DG // 2]], axis=-1)
    qv = (qp * cq[:, None, :] + rot * sq[:, None, :]) * scale  # [B, Hk, Dg]
    qv = qv.astype(np.float32)

    # folded cos/sin (k): ck = cos*w ; sk = sin*rot_w(w) with sign on 1st half
    ckw = cos_k * knorm_w[None, None, :]                       # [B, nb, Dg]
    skw = sin_k * _rot_w(knorm_w)[None, None, :]
    skw = skw.copy()
    skw[:, :, :DG // 2] *= -1.0

    # v table: v[b, h, blk, o] = ck[blk,o]*qv[h,o] + swap_halves(sk[blk]*qv[h])
    # so that  score = sum_o kc[blk,o] * v[blk,o]  reproduces
    #          sum_o rope(kc)[blk,o] * qv[o].
    vt = (ckw[:, None, :, :] * qv[:, :, None, :])              # [B, Hk, nb, Dg]
    su = skw[:, None, :, :] * qv[:, :, None, :]
    vt = vt + np.concatenate([su[..., DG // 2:], su[..., :DG // 2]], axis=-1)
    vt = vt.astype(np.float32)

    in_maps = []
    for core in range(N_CORES):
        b = core // 2
        h0 = (core % 2) * HEADS_PER_CORE
        heads = list(range(h0, h0 + HEADS_PER_CORE))
        im = {}
        for i, h in enumerate(heads):
            im["k%d" % i] = np.ascontiguousarray(k[b, :, h, :])
        # wk: [d, (h, 2, o)]; mean part scaled by 1/64
        wk_prep = np.empty((D, HEADS_PER_CORE, 2, DG), dtype=np.float32)
        for i, h in enumerate(heads):
            wk_prep[:, i, 0, :] = Wk[h, :D, :] / BLOCK
            wk_prep[:, i, 1, :] = Wk[h, D:, :]
        im["wk"] = wk_prep.reshape(D, HEADS_PER_CORE * 2 * DG)
        # v table: [p, (h, chunk, o)] with p = block index within chunk
        v_prep = np.empty((128, HEADS_PER_CORE, N_CHUNKS, DG), dtype=np.float32)
        for i, h in enumerate(heads):
            for cc in range(N_CHUNKS):
                v_prep[:, i, cc, :] = vt[b, h, cc * CHUNK_BLOCKS:(cc + 1) * CHUNK_BLOCKS, :]
        im["vtab"] = v_prep.reshape(128, HEADS_PER_CORE * N_CHUNKS * DG)
        # mask folds in row layout [h, nb]
        amr_prep = am[b, heads, :].astype(np.float32)
        im["amr"] = amr_prep
        im["mtr"] = ((1.0 - amr_prep) * NEG_MASK).astype(np.float32)
        im["idn"] = idn_np
        in_maps.append(im)

    res = run_bass_kernel_spmd(nc, in_maps, core_ids=list(range(N_CORES)),
                               trace=bool(int(os.environ.get("ATTNGATE_TRACE", "0"))))
    kernel.last_result = res

    sel = np.zeros((B, HK, NB), dtype=bool)
    for core in range(N_CORES):
        b = core // 2
        h0 = (core % 2) * HEADS_PER_CORE
        sel[b, h0:h0 + HEADS_PER_CORE, :] = res.results[core]["out_mask"] != 0.0
    mask = sel & am
    mask[:, :, -1] = True
    return mask


# revision 68
# speedup vs baseline: 1.0301x; 1.0045x over previous
"""AttnGate sparse-attention block-mask kernel for 8 Trainium2 NeuronCores.

Takes the full unsharded inputs, shards batch x k-head-group across the 8
cores (core c -> batch c//2, k-heads (c%2)*4..+4), runs one SPMD Bass kernel,
and gathers the full [B, Hk, nb] boolean block mask.

v2 structure (vs the v1 baseline):
  - all phase-2 math is done per 128-block chunk in [block, o] layout
    (blocks on partitions), so the rms scale is a per-partition scalar and
    the score is one fused multiply-accumulate.
  - rope and the q path are folded on the host into a single per-(head,
    block) table v[blk, o]; on device the score is
    rsq[blk] * sum_o kc[blk, o] * v[blk, o]  (one scalar_tensor_tensor).
  - max pooling: one DVE tensor_reduce over the in-block axis (f) via a
    transposed 3D access pattern. mean pooling: 64 PSUM-accumulated PE
    transposes (1/64 folded into Wk).
  - scores stream into sc_all[4, 1024]; a merge-chain top-k (extract the
    top-`budget` of running candidates U fresh slabs at chunks 3,5,6,7, all
    4 heads batched on partitions) runs overlapped with the stream; the
    final mask is a single is_ge against the per-head threshold (the
    budget-th largest score).
"""

import json
import math
import os
import sys

import numpy as np

sys.path.insert(0, "/opt/trn_rl_repo")

B, S, HK, D = 4, 65536, 8, 128
BLOCK = 64
NB = S // BLOCK          # 1024 blocks
DG = 128
HQ, G = 32, 4
N_CORES = 8
HEADS_PER_CORE = HK // 2  # 4
CHUNK_BLOCKS = 128        # blocks per pipeline chunk
N_CHUNKS = NB // CHUNK_BLOCKS  # 8
POS_PER_CHUNK = CHUNK_BLOCKS * BLOCK  # 8192 tokens
NEG_MASK = -1e20
SENTINEL = -1e30
EPS = 1e-6
USE_REDUCE_MAX = True

_compiled = {}


# ---------------------------------------------------------------------------
# walrus wait-capacity shim: split multi-wait instructions into single-wait
# NoOp carriers on the same engine (this walrus build accepts one sync wait
# per TPB instruction struct on the failing paths).
# ---------------------------------------------------------------------------
def _split_waits_json(bir_json):
    j = json.loads(bir_json.decode() if isinstance(bir_json, (bytes, bytearray)) else bir_json)
    n = 0
    for f in j.get("functions", []):
        for blk in f.get("blocks", []):
            out = []
            for inst in blk.get("instructions", []):
                si = inst.get("sync_info")
                waits = si.get("on_wait", []) if si else []
                if len(waits) > 1 and inst.get("engine") not in (None, "Unassigned"):
                    for w in waits[:-1]:
                        n += 1
                        out.append({
                            "debug": inst.get("debug", 0),
                            "engine": inst["engine"],
                            "ins": [], "outs": [],
                            "name": "WC-%d" % n,
                            "opcode": "NoOp",
                            "sync_info": {"on_update": [], "on_wait": [w]},
                        })
                    si["on_wait"] = waits[-1:]
                out.append(inst)
            blk["instructions"] = out
    return json.dumps(j).encode()


def _install_waitfix():
    import concourse.bass_utils as bu
    import concourse.bass2jax as b2j
    if getattr(bu, "_attngate_waitfix", False):
        return
    orig = bu.compile_bir_kernel

    def patched(bir_json, tmpdir, neff_name="file.neff"):
        return orig(_split_waits_json(bir_json), tmpdir, neff_name)

    bu.compile_bir_kernel = patched
    b2j.compile_bir_kernel = patched
    bu._attngate_waitfix = True


# ---------------------------------------------------------------------------
# device program
# ---------------------------------------------------------------------------
def _build_program(n_rounds):
    import concourse.bass as bass
    import concourse.mybir as mybir
    from concourse import tile
    from contextlib import ExitStack

    dt = mybir.dt
    f32 = dt.float32
    ALU = mybir.AluOpType
    ACT = mybir.ActivationFunctionType

    budget = n_rounds * 8

    nc = bass.Bass()

    k_d = [nc.dram_tensor("k%d" % h, [S, D], f32, kind="ExternalInput")
           for h in range(HEADS_PER_CORE)]
    wk_d = nc.dram_tensor("wk", [D, HEADS_PER_CORE * 2 * DG], f32, kind="ExternalInput")
    v_d = nc.dram_tensor("vtab", [128, HEADS_PER_CORE * N_CHUNKS * DG], f32, kind="ExternalInput")
    amr_d = nc.dram_tensor("amr", [HEADS_PER_CORE, NB], f32, kind="ExternalInput")
    mtr_d = nc.dram_tensor("mtr", [HEADS_PER_CORE, NB], f32, kind="ExternalInput")
    idn_d = nc.dram_tensor("idn", [128, 128], f32, kind="ExternalInput")
    out_d = nc.dram_tensor("out_mask", [HEADS_PER_CORE, NB], f32, kind="ExternalOutput")

    steps = [(c, h) for c in range(N_CHUNKS) for h in range(HEADS_PER_CORE)]
    NS = len(steps)
    # merge schedule: extract top-budget of slabs 0..3 after chunk 3, fold
    # slabs 4..5 in after chunk 5 (both trickled), join 6..7 at the end.
    MERGE1_C, MERGE2_C = 3, 5

    with tile.TileContext(nc) as tc, ExitStack() as ctx:
        consts = ctx.enter_context(tc.tile_pool(name="consts", bufs=1))
        chunks = ctx.enter_context(tc.tile_pool(name="chunks", bufs=3))
        trees = ctx.enter_context(tc.tile_pool(name="trees", bufs=1))
        pooled = ctx.enter_context(tc.tile_pool(name="pooled", bufs=2))
        ph2 = ctx.enter_context(tc.tile_pool(name="ph2", bufs=2))
        ph3 = ctx.enter_context(tc.tile_pool(name="ph3", bufs=3))
        candp = ctx.enter_context(tc.tile_pool(name="candp", bufs=1))
        psA_p = ctx.enter_context(tc.tile_pool(name="psA", bufs=2, space="PSUM"))
        psM_p = ctx.enter_context(tc.tile_pool(name="psM", bufs=2, space="PSUM"))
        psKC_p = ctx.enter_context(tc.tile_pool(name="psKC", bufs=2, space="PSUM"))
        psS_p = ctx.enter_context(tc.tile_pool(name="psS", bufs=2, space="PSUM"))

        kts = {}
        def emit_dma(i):
            c, h = steps[i]
            kt = chunks.tile([128, BLOCK, D], f32, tag="kt")
            nc.sync.dma_start(
                kt[:, :, :],
                k_d[h][c * POS_PER_CHUNK:(c + 1) * POS_PER_CHUNK, :]
                .rearrange("(p f) d -> p f d", p=128))
            kts[i] = kt

        # first chunk DMA goes out first; consts trickle in behind it in
        # order of first use so they do not steal bandwidth from chunk 0.
        emit_dma(0)
        idn = consts.tile([128, 128], f32)
        nc.sync.dma_start(idn[:], idn_d[:, :])
        emit_dma(1)
        wk = consts.tile([128, HEADS_PER_CORE * 2 * DG], f32)
        nc.sync.dma_start(wk[:], wk_d[:, :])
        emit_dma(2)
        amr = consts.tile([HEADS_PER_CORE, NB], f32)
        nc.sync.dma_start(amr[:], amr_d[:, :])
        mtr = consts.tile([HEADS_PER_CORE, NB], f32)
        nc.sync.dma_start(mtr[:], mtr_d[:, :])
        vtab = consts.tile([128, HEADS_PER_CORE * N_CHUNKS * DG], f32)

        sc_all = consts.tile([HEADS_PER_CORE, NB], f32)
        epsc = consts.tile([128, 1], f32)
        nc.vector.memset(epsc[:], EPS)

        state = {}
        s4_by_c = {}
        merge_backlog = []
        merge_M = [None, None]

        def emit_front(i):
            c, h = steps[i]
            kt = kts[i]
            # max pool over f -> [blk, d]: pairwise tree, contiguous slices
            prev, size = kt, BLOCK
            lvl = 0
            while size > 1:
                half = size // 2
                pool = trees if half > 1 else pooled
                nxt = pool.tile([128, half, D], f32, tag="tr%d" % lvl)
                nc.vector.tensor_tensor(nxt[:, :, :], prev[:, 0:half, :],
                                        prev[:, half:size, :], ALU.max)
                prev, size = nxt, half
                lvl += 1
            # mean: 64 accumulated PE transposes -> [d, blk]
            psA = psA_p.tile([128, 128], f32, tag="psA")
            for f in range(BLOCK):
                nc.tensor.matmul(psA[:], kt[:, f, :], idn[:],
                                 is_transpose=True,
                                 start=(f == 0), stop=(f == BLOCK - 1))
            meanT = pooled.tile([128, 128], f32, tag="meanT")
            nc.scalar.copy(meanT[:], psA[:])
            state[("front", i)] = (prev, meanT)

        def emit_mid(i):
            c, h = steps[i]
            mxP, meanT = state.pop(("front", i))
            if h == 0:
                # sr[:, 0:4] = unscaled score columns, sr[:, 4:8] = rms roots
                s4_by_c[c] = ph3.tile([128, 2 * HEADS_PER_CORE], f32, tag="sr",
                                      name="sr")
            sr = s4_by_c[c]
            psM = psM_p.tile([128, 128], f32, tag="psM")
            nc.tensor.matmul(psM[:], mxP[:, 0, :], idn[:],
                             is_transpose=True, start=True, stop=True)
            maxT = pooled.tile([128, 128], f32, tag="maxT")
            nc.scalar.copy(maxT[:], psM[:])
            # projection: kc[blk, o] (pooled_T as stationary)
            psKC = psKC_p.tile([128, 128], f32, tag="psKC")
            nc.tensor.matmul(psKC[:], meanT[:],
                             wk[:, (h * 2) * DG:(h * 2 + 1) * DG],
                             start=True, stop=False)
            nc.tensor.matmul(psKC[:], maxT[:],
                             wk[:, (h * 2 + 1) * DG:(h * 2 + 2) * DG],
                             start=False, stop=True)
            kc = ph3.tile([128, 128], f32, tag="kc")
            nc.scalar.copy(kc[:], psKC[:])
            # rms root: sr[:, 4+h] = sqrt(mean(kc^2) + eps), all on Act
            sq = ph2.tile([128, 128], f32, tag="sq")
            rss = ph2.tile([128, 1], f32, tag="rss")
            nc.scalar.activation(sq[:], psKC[:], ACT.Square, accum_out=rss[:])
            r2 = ph2.tile([128, 1], f32, tag="r2")
            nc.scalar.activation(r2[:], rss[:], ACT.Identity,
                                 bias=epsc[:], scale=1.0 / DG)
            nc.scalar.activation(sr[:, 4 + h:5 + h], r2[:], ACT.Sqrt)
            state[i] = kc

        def emit_back(i):
            c, h = steps[i]
            kc = state.pop(i)
            sr = s4_by_c[c]
            vi = (h * N_CHUNKS + c) * DG
            j1 = ph2.tile([128, 128], f32, tag="j1")
            nc.vector.scalar_tensor_tensor(j1[:], kc[:], 1.0,
                                           vtab[:, vi:vi + DG],
                                           ALU.mult, ALU.mult,
                                           accum_out=sr[:, h:h + 1])

        def emit_chunk_back_a(c):
            # transpose scores+roots, start the divide
            sr = s4_by_c.pop(c)
            psS = psS_p.tile([HEADS_PER_CORE, 256], f32, tag="psS")
            nc.tensor.matmul(psS[:, 0:128], sr[:, 0:4], idn[:],
                             is_transpose=True, start=True, stop=True)
            nc.tensor.matmul(psS[:, 128:256], sr[:, 4:8], idn[:],
                             is_transpose=True, start=True, stop=True)
            rr = ph2.tile([HEADS_PER_CORE, 128], f32, tag="rr")
            nc.vector.reciprocal(rr[:], psS[:, 128:256])
            state[("psS", c)] = (psS, rr)

        def emit_chunk_back(c):
            # finish: divide, mask, store the score rows
            psS, rr = state.pop(("psS", c))
            srow = ph2.tile([HEADS_PER_CORE, 128], f32, tag="srow")
            nc.vector.tensor_tensor(srow[:], psS[:, 0:128], rr[:], ALU.mult)
            sl = slice(c * CHUNK_BLOCKS, (c + 1) * CHUNK_BLOCKS)
            nc.vector.tensor_tensor(srow[:], srow[:], amr[:, sl], ALU.mult)
            nc.vector.tensor_tensor(srow[:], srow[:], mtr[:, sl], ALU.add)
            nc.scalar.copy(sc_all[:, sl], srow[:])
            if c == MERGE1_C:
                queue_merge(None, 0, c + 1)
            elif c == MERGE2_C:
                # defer until merge1's rounds are fully emitted
                merge_backlog.append(
                    lambda: queue_merge(merge_M[0], MERGE1_C + 1, MERGE2_C + 1))

        def queue_merge(prev_M, c_lo, c_hi):
            # extract top-budget of [prev_M | slabs c_lo..c_hi-1], trickled
            nslab = (c_hi - c_lo) * CHUNK_BLOCKS
            mw = (budget if prev_M is not None else 0) + nslab
            midx = 0 if prev_M is None else 1
            work = candp.tile([HEADS_PER_CORE, mw], f32, tag="w%d" % midx,
                              name="mwork")
            off = 0
            if prev_M is not None:
                nc.scalar.copy(work[:, 0:budget], prev_M[:])
                off = budget
            nc.scalar.copy(work[:, off:off + nslab],
                           sc_all[:, c_lo * CHUNK_BLOCKS:c_hi * CHUNK_BLOCKS])
            M = candp.tile([HEADS_PER_CORE, budget], f32, tag="c%d" % midx,
                           name="mcand")
            merge_M[midx] = M

            def round_thunk(r):
                def go():
                    m8 = M[:, r * 8:(r + 1) * 8]
                    nc.vector.max(m8, work[:])
                    nc.vector.match_replace(work[:], m8, work[:], SENTINEL)
                return go
            for r in range(n_rounds):
                merge_backlog.append(round_thunk(r))

        # ---- software-pipelined emission --------------------------------
        CB_SKEW = 6   # chunk-back for chunk c runs at slots 4c+CB_SKEW (+1)
        for slot in range(NS + CB_SKEW + 2):
            if 3 <= slot < NS:
                emit_dma(slot)
            if slot == 2:
                nc.sync.dma_start(vtab[:], v_d[:, :])
            if slot < NS:
                emit_front(slot)
            if 1 <= slot <= NS:
                emit_mid(slot - 1)
            if 2 <= slot <= NS + 1:
                emit_back(slot - 2)
            if slot >= CB_SKEW and (slot - CB_SKEW) % 4 == 0:
                cb = (slot - CB_SKEW) // 4
                if cb < N_CHUNKS:
                    emit_chunk_back_a(cb)
            if slot >= CB_SKEW + 1 and (slot - CB_SKEW - 1) % 4 == 0:
                cb = (slot - CB_SKEW - 1) // 4
                if cb < N_CHUNKS:
                    emit_chunk_back(cb)
            npop = 2 if len(merge_backlog) > (NS + CB_SKEW + 1 - slot) else 1
            for _ in range(npop):
                if merge_backlog:
                    merge_backlog.pop(0)()

        while merge_backlog:
            merge_backlog.pop(0)()

        # ---- final merge: [merge_M[1] | slabs 6..7] ---------------------
        rem = (N_CHUNKS - MERGE2_C - 1) * CHUNK_BLOCKS
        fw = budget + rem
        fwork = candp.tile([HEADS_PER_CORE, fw], f32, tag="wfin")
        nc.scalar.copy(fwork[:, 0:budget], merge_M[1][:])
        nc.scalar.copy(fwork[:, budget:fw],
                       sc_all[:, (MERGE2_C + 1) * CHUNK_BLOCKS:NB])
        fM = candp.tile([HEADS_PER_CORE, budget], f32, tag="cfin")
        for r in range(n_rounds):
            m8 = fM[:, r * 8:(r + 1) * 8]
            nc.vector.max(m8, fwork[:])
            nc.vector.match_replace(fwork[:], m8, fwork[:], SENTINEL)

        # ---- final threshold + mask ------------------------------------
        t_ap = fM[:, budget - 1:budget]
        maskf = consts.tile([HEADS_PER_CORE, NB], f32)
        nc.vector.tensor_scalar(maskf[:], sc_all[:], t_ap, None, ALU.is_ge)
        nc.sync.dma_start(out_d[:, :], maskf[:])

    return nc


def _rot_w(w):
    return np.concatenate([w[DG // 2:], w[:DG // 2]])


def _rmsnorm_np(x, w, eps=1e-6):
    v = np.mean(np.square(x), axis=-1, keepdims=True)
    return x / np.sqrt(v + eps) * w


def kernel(k, q, Wq, Wk, qnorm_w, knorm_w, cos_q, sin_q, cos_k, sin_k,
           attention_mask, block_budget):
    _install_waitfix()
    from concourse.bass_utils import run_bass_kernel_spmd

    k = np.asarray(k, dtype=np.float32)
    q = np.asarray(q, dtype=np.float32)
    Wq = np.asarray(Wq, dtype=np.float32)
    Wk = np.asarray(Wk, dtype=np.float32)
    qnorm_w = np.asarray(qnorm_w, dtype=np.float32)
    knorm_w = np.asarray(knorm_w, dtype=np.float32)
    cos_q = np.asarray(cos_q, dtype=np.float32)
    sin_q = np.asarray(sin_q, dtype=np.float32)
    cos_k = np.asarray(cos_k, dtype=np.float32)
    sin_k = np.asarray(sin_k, dtype=np.float32)
    am = np.asarray(attention_mask).astype(bool)
    budget = int(block_budget)
    assert budget % 8 == 0 and 0 < budget <= 512
    n_rounds = budget // 8

    scale = 1.0 / math.sqrt(DG)

    key = (n_rounds,)
    if key not in _compiled:
        _compiled[key] = _build_program(n_rounds)
    nc = _compiled[key]

    idn_np = np.eye(128, dtype=np.float32)

    # ---- host q path: qv = rope(rmsnorm(q @ Wq)) * scale ----------------
    qg = q[:, 0].reshape(B, HK, G, D)
    qp = np.einsum('bkgi,kgio->bko', qg, Wq)
    qp = _rmsnorm_np(qp, qnorm_w)
    cq = cos_q[:, 0]  # [B, Dg]
    sq = sin_q[:, 0]
    rot = np.concatenate([-qp[..., DG // 2:], qp[..., :DG // 2]], axis=-1)
    qv = (qp * cq[:, None, :] + rot * sq[:, None, :]) * scale  # [B, Hk, Dg]
    qv = qv.astype(np.float32)

    # folded cos/sin (k): ck = cos*w ; sk = sin*rot_w(w) with sign on 1st half
    ckw = cos_k * knorm_w[None, None, :]                       # [B, nb, Dg]
    skw = sin_k * _rot_w(knorm_w)[None, None, :]
    skw = skw.copy()
    skw[:, :, :DG // 2] *= -1.0

    # v table: v[b, h, blk, o] = ck[blk,o]*qv[h,o] + swap_halves(sk[blk]*qv[h])
    # so that  score = sum_o kc[blk,o] * v[blk,o]  reproduces
    #          sum_o rope(kc)[blk,o] * qv[o].
    vt = (ckw[:, None, :, :] * qv[:, :, None, :])              # [B, Hk, nb, Dg]
    su = skw[:, None, :, :] * qv[:, :, None, :]
    vt = vt + np.concatenate([su[..., DG // 2:], su[..., :DG // 2]], axis=-1)
    vt = vt.astype(np.float32)

    in_maps = []
    for core in range(N_CORES):
        b = core // 2
        h0 = (core % 2) * HEADS_PER_CORE
        heads = list(range(h0, h0 + HEADS_PER_CORE))
        im = {}
        for i, h in enumerate(heads):
            im["k%d" % i] = np.ascontiguousarray(k[b, :, h, :])
        # wk: [d, (h, 2, o)]; mean part scaled by 1/64
        wk_prep = np.empty((D, HEADS_PER_CORE, 2, DG), dtype=np.float32)
        for i, h in enumerate(heads):
            wk_prep[:, i, 0, :] = Wk[h, :D, :] / BLOCK
            wk_prep[:, i, 1, :] = Wk[h, D:, :]
        im["wk"] = wk_prep.reshape(D, HEADS_PER_CORE * 2 * DG)
        # v table: [p, (h, chunk, o)] with p = block index within chunk
        v_prep = np.empty((128, HEADS_PER_CORE, N_CHUNKS, DG), dtype=np.float32)
        for i, h in enumerate(heads):
            for cc in range(N_CHUNKS):
                v_prep[:, i, cc, :] = vt[b, h, cc * CHUNK_BLOCKS:(cc + 1) * CHUNK_BLOCKS, :]
        im["vtab"] = v_prep.reshape(128, HEADS_PER_CORE * N_CHUNKS * DG)
        # mask folds in row layout [h, nb]
        amr_prep = am[b, heads, :].astype(np.float32)
        im["amr"] = amr_prep
        im["mtr"] = ((1.0 - amr_prep) * NEG_MASK).astype(np.float32)
        im["idn"] = idn_np
        in_maps.append(im)

    res = run_bass_kernel_spmd(nc, in_maps, core_ids=list(range(N_CORES)),
                               trace=bool(int(os.environ.get("ATTNGATE_TRACE", "0"))))
    kernel.last_result = res

    sel = np.zeros((B, HK, NB), dtype=bool)
    for core in range(N_CORES):
        b = core // 2
        h0 = (core % 2) * HEADS_PER_CORE
        sel[b, h0:h0 + HEADS_PER_CORE, :] = res.results[core]["out_mask"] != 0.0
    mask = sel & am
    mask[:, :, -1] = True
    return mask
